# revision 1
# baseline (speedup 1.0000x reference)
"""Trainium2 Bass kernel for nn_MemoryAccess (scatter_memory).

Strategy (8 NeuronCores, SPMD):
  Launch 1 (8 cores): FA_r replicated (exact fp32); W_read column-sharded
    3750 cols/core, scored in bf16.  Returns scores z and the exact fp32
    FA_r output `a`.
  Host glue: combine shards, take top-8 candidates per (b, read_slot), rescore
    them exactly in fp64 using `a` (immune to bf16 scoring noise -> argmax and
    max are exact), tanh, gather the 48 selected memory rows (resharding glue).
  Launch 2 (1 core): update-gate branch + the 3-step recurrence.  FA_um is
    batched over the 3 independent steps as 48 rows.  Weights that only reach
    the output through the heavily-diluted r2 gate (uw*, umw*, wum) are bf16;
    the direct output path (amw*, wam) stays fp32r.  The sequential FA_am
    chain works in transposed layout (fT = catT * gT); per-step y PSUMs are
    pre-accumulated off-chain so only the m-dependent matmuls are serial.

Numerical notes:
  - softmax without max-subtraction (|y| < ~4 here, mathematically identical),
    1/sum folded through the second matmul, off the serial critical path.
  - biases enter matmul PSUMs via a ones-row matmul (lhsT=[1,p] ones,
    rhs=[1,n] bias) so activations read PSUM directly - no DVE add hop.
    All biases arrive in ONE packed DMA; partition-broadcasts are built with
    K=1 ones-matmuls on PE instead of DMAs (the DMA queue costs ~625ns/DMA).
  - sigmoid(x) == 0.5*(1+tanh(x/2)) keeps every ACT func in one table set.
  - m = read_w*relu(t) == relu(read_w*t) since read_w=tanh(max z)>0; fused
    into one ACT op via activation(Relu, scale=read_w).
  - dependency-free bf16 filler matmuls keep the PE p-state warm across the
    softmax hops of launch 1's fp32 FA chain.

kernel(**inputs) takes FULL inputs, returns the FULL [16, 256] output.
"""

import numpy as np
import ml_dtypes

import concourse.bass as bass
import concourse.tile as tile
import concourse.mybir as mybir
from concourse import bacc, bass_utils
from concourse.masks import make_identity

F32 = mybir.dt.float32
F32R = mybir.dt.float32r
BF16 = mybir.dt.bfloat16
AX = mybir.AxisListType
AF = mybir.ActivationFunctionType
ALU = mybir.AluOpType

B, IN_CH, SLOTS, SLOT_SIZE, READ_SLOTS = 16, 512, 10000, 256, 3
N_CORES = 8
TOTAL_COLS = READ_SLOTS * SLOTS          # 30000
SHARD = TOTAL_COLS // N_CORES            # 3750
MMF = 512                                # max moving free dim / PSUM f32 bank


def _chunks(total, step=MMF):
    for c0 in range(0, total, step):
        yield c0, min(step, total - c0)


def _emit_transposes(nc, tpp, persist, src_sb, p, c, identity, name,
                     out_dtype=F32, alt=0):
    """src_sb [p, c] -> SBUF tile [128, kt, p] holding src^T k-tiles.

    All kt transposes land in ONE psum bank, then ONE copy moves them to SBUF
    (one engine hop instead of kt).  Returns the list of [128, p] slices.
    """
    kt = c // 128
    pst = tpp.tile([128, 6, 48], F32, tag="tps")
    for k in range(kt):
        nc.tensor.transpose(pst[:, k, :p], src_sb[:, k * 128:(k + 1) * 128],
                            identity[:p, :p])
    sb = persist.tile([128, kt, p], out_dtype, tag=f"{name}_T")
    if alt:
        nc.scalar.copy(sb[:, :, :], pst[:, :kt, :p])
    else:
        nc.vector.tensor_copy(sb[:, :, :], pst[:, :kt, :p])
    return [sb[:, k, :] for k in range(kt)]


def _mm_bias(nc, ps, lhsT_tiles, rhs_sb, p, c0, w, ones, bias_row,
             stop=True):
    """ps[:p,:w] = sum_k lhsT_k.T @ rhs[:,k,c0:c0+w] + ones.T @ bias_row."""
    kt = len(lhsT_tiles)
    for k in range(kt):
        nc.tensor.matmul(ps[:p, :w], lhsT_tiles[k],
                         rhs_sb[:, k, c0:c0 + w],
                         start=(k == 0), stop=False)
    nc.tensor.matmul(ps[:p, :w], ones[:, :p], bias_row[:, c0:c0 + w],
                     start=False, stop=stop)


def _emit_fa(nc, pools, x_sb, p, c, w1_sb, b1r, w2_sb, b2b, identity, ones,
             name, mm_dt=F32, fillers=(0, 0, 0, 0)):
    """FastAttention without max-subtraction, 1/sum folded into the output:
         e = exp(x@w1 + b1);  rs = 1/sum(e);  out = x * (rs*(e@w2) + b2)
    b1 enters the PSUM via the ones-row matmul; Exp reads PSUM directly.
    """
    persist, psum, tpp, fpp = pools
    xT = _emit_transposes(nc, tpp, persist, x_sb, p, c, identity, f"{name}x",
                          out_dtype=mm_dt)
    _fill(nc, fpp, fillers[0])

    e = persist.tile([p, c], F32, tag=f"{name}_e")
    for c0, w in _chunks(c):
        ps = psum.tile([48, MMF], F32, tag="mm")
        _mm_bias(nc, ps, xT, w1_sb, p, c0, w, ones, b1r)
        nc.scalar.activation(e[:, c0:c0 + w], ps[:p, :w], AF.Exp)
    _fill(nc, fpp, fillers[1])

    ssum = persist.tile([p, 1], F32, tag=f"{name}_sum")
    nc.vector.reduce_sum(out=ssum[:, :], in_=e[:, :], axis=AX.X)
    rs = persist.tile([p, 1], F32, tag=f"{name}_rs")
    nc.vector.reciprocal(rs[:, :], ssum[:, :])

    eT = _emit_transposes(nc, tpp, persist, e, p, c, identity, f"{name}e",
                          out_dtype=mm_dt, alt=1)
    _fill(nc, fpp, fillers[2])
    out = persist.tile([p, c], F32, tag=f"{name}_o")
    for c0, w in _chunks(c):
        ps = psum.tile([48, MMF], F32, tag="mm")
        kt = len(eT)
        for k in range(kt):
            nc.tensor.matmul(ps[:p, :w], eT[k], w2_sb[:, k, c0:c0 + w],
                             start=(k == 0), stop=(k == kt - 1))
        sl = slice(c0, c0 + w)
        nc.vector.scalar_tensor_tensor(out[:, sl], ps[:p, :w], rs[:, :],
                                       b2b[:, sl], op0=ALU.mult, op1=ALU.add)
        nc.vector.tensor_mul(out[:, sl], out[:, sl], x_sb[:, sl])
    _fill(nc, fpp, fillers[3])
    return out


_FILL_STATE = {}


def _fill(nc, fpp, n):
    """Dependency-free bf16 matmuls that keep the PE p-state warm while other
    engines run serial softmax hops."""
    if not n or fpp is None:
        return
    fs = _FILL_STATE.get(id(nc))
    if fs is None:
        return
    for _ in range(n):
        ps = fpp.tile([64, MMF], F32, tag="fill")
        nc.tensor.matmul(ps[:, :], fs[:, :64], fs[:, 64:64 + MMF],
                         start=True, stop=True)


def _load_w(nc, persist, ap, kt, n, name, dtype=F32, split=False):
    """DRAM [kt*128, n] -> SBUF [128, kt, n]."""
    sb = persist.tile([128, kt, n], dtype, tag=name)
    src = ap.rearrange("(k p) n -> p k n", p=128)
    if split:
        for k in range(kt):
            nc.sync.dma_start(out=sb[:, k:k + 1, :], in_=src[:, k:k + 1, :])
    else:
        nc.sync.dma_start(out=sb[:, :, :], in_=src[:, :, :])
    return sb


def _load_b(nc, persist, ap, p, n, name):
    """DRAM [n] -> SBUF [p, n] broadcast across partitions."""
    sb = persist.tile([p, n], F32, tag=name)
    nc.sync.dma_start(out=sb[:, :], in_=bass.AP(
        tensor=ap.tensor, offset=ap.offset, ap=[[0, p]] + list(ap.ap)))
    return sb


def _load_pack(nc, persist, ap, n, name):
    """DRAM [1, n] packed-bias tensor -> SBUF [1, n] row (one DMA)."""
    sb = persist.tile([1, n], F32R, tag=name)
    nc.sync.dma_start(out=sb[:, :], in_=ap[:, :])
    return sb


def _bias_bcast(nc, pools, row, p, c, name, ones):
    """[p, c] broadcast of a bias row via K=1 ones-matmuls (no DMA)."""
    persist, psum, tpp, fpp = pools
    out = persist.tile([p, c], F32, tag=name)
    for c0, w in _chunks(c):
        ps = psum.tile([48, MMF], F32, tag="mm")
        nc.tensor.matmul(ps[:p, :w], ones[:, :p], row[:, c0:c0 + w],
                         start=True, stop=True)
        nc.vector.tensor_copy(out[:, c0:c0 + w], ps[:p, :w])
    return out


def _prelude(nc, persist, fill=False):
    identity = persist.tile([64, 64], F32, tag="ident")
    make_identity(nc, identity[:, :])
    ones_f = persist.tile([1, 48], F32, tag="ones_f")
    nc.vector.memset(ones_f[:, :], 1.0)
    ones = persist.tile([1, 48], F32R, tag="ones")
    nc.scalar.copy(ones[:, :], ones_f[:, :])
    # Touch Exp early so the LoadActFuncSet overlaps the initial DMAs.
    t = persist.tile([1, 2], F32, tag="actwarm")
    nc.vector.memset(t[:, :], 0.0)
    nc.scalar.activation(t[:, :], t[:, :], AF.Exp)
    if fill:
        fs = persist.tile([128, 64 + MMF], BF16, tag="fillsrc")
        nc.vector.memset(fs[:, :], 0.0)
        _FILL_STATE[id(nc)] = fs
    return identity, ones


def _build_launch1():
    nc = bacc.Bacc("TRN2", target_bir_lowering=False, debug=False,
                   num_devices=N_CORES)
    d = {}
    for name, shape, dt in [
        ("x", [B, IN_CH], F32),
        ("rw1", [IN_CH, IN_CH], F32),
        ("rw2", [IN_CH, IN_CH], F32),
        ("bpk", [1, 2 * IN_CH], F32R),
        ("wsh", [IN_CH, SHARD], BF16), ("bsh", [SHARD], F32),
    ]:
        d[name] = nc.dram_tensor(name, shape, dt, kind="ExternalInput").ap()
    z_out = nc.dram_tensor("z", [B, SHARD], F32, kind="ExternalOutput").ap()
    a_out = nc.dram_tensor("a", [B, IN_CH], F32, kind="ExternalOutput").ap()

    with tile.TileContext(nc) as tc:
        with (
            tc.tile_pool(name="persist", bufs=1) as persist,
            tc.tile_pool(name="psum", bufs=2, space="PSUM") as psum,
            tc.tile_pool(name="tpp", bufs=2, space="PSUM") as tpp,
            tc.tile_pool(name="fpp", bufs=2, space="PSUM") as fpp,
        ):
            pools = (persist, psum, tpp, fpp)
            identity, ones = _prelude(nc, persist, fill=True)

            x_sb = persist.tile([B, IN_CH], F32, tag="x")
            nc.sync.dma_start(out=x_sb[:, :], in_=d["x"][:, :])

            rw1 = _load_w(nc, persist, d["rw1"], 4, IN_CH, "rw1", split=True)
            rw2 = _load_w(nc, persist, d["rw2"], 4, IN_CH, "rw2", split=True)
            bpk = _load_pack(nc, persist, d["bpk"], 2 * IN_CH, "bpk")
            rb1 = bpk[:, 0:IN_CH]
            rb2 = _bias_bcast(nc, pools, bpk[:, IN_CH:2 * IN_CH], B, IN_CH,
                              "rb2b", ones)

            wsh = persist.tile([128, 4, SHARD], BF16, tag="wsh")
            wsh_src = d["wsh"].rearrange("(k p) n -> p k n", p=128)
            first = True
            for c0, w in _chunks(SHARD, 1250):
                nc.sync.dma_start(out=wsh[:, :, c0:c0 + w],
                                  in_=wsh_src[:, :, c0:c0 + w])
                if first:
                    bshb = _load_b(nc, persist, d["bsh"], B, SHARD, "bshb")
                    first = False

            # a = FA_r(x)  (exact fp32; feeds argmax + host rescore)
            a_sb = _emit_fa(nc, pools, x_sb, B, IN_CH, rw1, rb1, rw2, rb2,
                            identity, ones, "far", fillers=(6, 12, 1, 7))
            nc.sync.dma_start(out=a_out[:, :], in_=a_sb[:, :])
            aT = _emit_transposes(nc, tpp, persist, a_sb, B, IN_CH, identity,
                                  "aT", out_dtype=BF16)

            # z = a @ W_shard + b_shard  (bf16; host rescores the top-8)
            z_sb = persist.tile([B, SHARD], F32, tag="z")
            for c0, w in _chunks(SHARD):
                ps = psum.tile([48, MMF], F32, tag="mm")
                for k in range(4):
                    nc.tensor.matmul(ps[:B, :w], aT[k],
                                     wsh[:, k, c0:c0 + w],
                                     start=(k == 0), stop=(k == 3))
                nc.vector.tensor_add(z_sb[:, c0:c0 + w], ps[:B, :w],
                                     bshb[:, c0:c0 + w])
            nc.sync.dma_start(out=z_out[:, :], in_=z_sb[:, :])

    nc.compile()
    return nc


def _build_launch2():
    nc = bacc.Bacc("TRN2", target_bir_lowering=False, debug=False, num_devices=1)
    CU, CA = IN_CH + SLOT_SIZE, 2 * SLOT_SIZE   # 768, 512
    P3 = READ_SLOTS * B                          # 48
    d = {}
    BF16_NAMES = ("uw1", "uw2", "umw1", "umw2", "wum")
    F32R_NAMES = ("amw1", "amw2", "wam")
    # bias pack layout (offsets into bpk):
    # ub1@0, ub2@512, umb1@1024, umb2@1792, bum@2560, amb1@2816, amb2@3328,
    # bam@3840, buw@4096, pad@4099 -> total 4100
    NB = 4100
    for name, shape in [
        ("x", [B, IN_CH]), ("rall", [B, READ_SLOTS, SLOT_SIZE]),
        ("readw", [B, READ_SLOTS]),
        ("bpk", [1, NB]),
        ("uw1", [IN_CH, IN_CH]),
        ("uw2", [IN_CH, IN_CH]),
        ("wuw", [IN_CH, READ_SLOTS]),
        ("umw1", [CU, CU]), ("umw2", [CU, CU]),
        ("wum", [CU, SLOT_SIZE]),
        ("amw1", [CA, CA]), ("amw2", [CA, CA]),
        ("wam", [CA, SLOT_SIZE]),
    ]:
        dt = BF16 if name in BF16_NAMES else (
            F32R if name in F32R_NAMES or name == "bpk" else F32)
        d[name] = nc.dram_tensor(name, shape, dt, kind="ExternalInput").ap()
    out_dram = nc.dram_tensor("out", [B, SLOT_SIZE], F32, kind="ExternalOutput").ap()

    with tile.TileContext(nc) as tc:
        with (
            tc.tile_pool(name="persist", bufs=1) as persist,
            tc.tile_pool(name="psum", bufs=2, space="PSUM") as psum,
            tc.tile_pool(name="tpp", bufs=2, space="PSUM") as tpp,
            tc.tile_pool(name="ypsum", bufs=2, space="PSUM") as ypsum,
        ):
            pools = (persist, psum, tpp, None)
            identity, ones = _prelude(nc, persist)

            # small inputs
            x_sb = persist.tile([B, IN_CH], F32, tag="x")
            nc.sync.dma_start(out=x_sb[:, :], in_=d["x"][:, :])
            cat3 = persist.tile([P3, CU], F32, tag="cat3")
            rall = d["rall"]
            nc.sync.dma_start(
                out=cat3[:, :SLOT_SIZE],
                in_=bass.AP(tensor=rall.tensor, offset=rall.offset,
                            ap=[[SLOT_SIZE, READ_SLOTS],
                                [READ_SLOTS * SLOT_SIZE, B], [1, SLOT_SIZE]]))
            nc.sync.dma_start(
                out=cat3[:, SLOT_SIZE:],
                in_=bass.AP(tensor=d["x"].tensor, offset=d["x"].offset,
                            ap=[[0, READ_SLOTS], [IN_CH, B], [1, IN_CH]]))
            readw_sb = persist.tile([B, READ_SLOTS], F32, tag="readw")
            nc.sync.dma_start(out=readw_sb[:, :], in_=d["readw"][:, :])

            # weight DMAs in consumption order; all biases from one packed
            # row DMA, partition-broadcasts via K=1 ones-matmuls on PE.
            bpk = _load_pack(nc, persist, d["bpk"], NB, "bpk")
            ub1 = bpk[:, 0:512]
            umb1 = bpk[:, 1024:1792]
            bumr = bpk[:, 2560:2816]
            amb1 = bpk[:, 2816:3328]
            bamr = bpk[:, 3840:4096]
            buwr = bpk[:, 4096:4100]
            uw1 = _load_w(nc, persist, d["uw1"], 4, IN_CH, "uw1", dtype=BF16)
            uw2 = _load_w(nc, persist, d["uw2"], 4, IN_CH, "uw2", dtype=BF16)
            wuw = _load_w(nc, persist, d["wuw"], 4, READ_SLOTS, "wuw")
            ub2 = _bias_bcast(nc, pools, bpk[:, 512:1024], B, IN_CH, "ub2b",
                              ones)
            umw1 = _load_w(nc, persist, d["umw1"], 6, CU, "umw1", dtype=BF16)
            umw2 = _load_w(nc, persist, d["umw2"], 6, CU, "umw2", dtype=BF16)
            umb2 = _bias_bcast(nc, pools, bpk[:, 1792:2560], P3, CU, "umb2b",
                               ones)
            wum = _load_w(nc, persist, d["wum"], 6, SLOT_SIZE, "wum",
                          dtype=BF16)
            amw1 = _load_w(nc, persist, d["amw1"], 4, CA, "amw1", dtype=F32R)
            amw2 = _load_w(nc, persist, d["amw2"], 4, CA, "amw2", dtype=F32R)
            amb2 = _bias_bcast(nc, pools, bpk[:, 3328:3840], B, CA, "amb2b",
                               ones)
            wam = _load_w(nc, persist, d["wam"], 4, SLOT_SIZE, "wam",
                          dtype=F32R)

            # ---- update-gate branch: upd_w = sigmoid(FA_u(x) @ W_uw + b_uw)
            au = _emit_fa(nc, pools, x_sb, B, IN_CH, uw1, ub1, uw2, ub2,
                          identity, ones, "fau", mm_dt=BF16)
            auT = _emit_transposes(nc, tpp, persist, au, B, IN_CH, identity,
                                   "auT")
            utp = psum.tile([48, MMF], F32, tag="mm")
            for k in range(4):
                nc.tensor.matmul(utp[:B, :READ_SLOTS], auT[k],
                                 wuw[:, k, :READ_SLOTS],
                                 start=(k == 0), stop=False)
            nc.tensor.matmul(utp[:B, :4], ones[:, :B], buwr[:, :4],
                             start=False, stop=True)
            # sigmoid(t) = 0.5*tanh(0.5*t) + 0.5  (keeps one ACT table set)
            uth = persist.tile([B, READ_SLOTS], F32, tag="uth")
            nc.scalar.activation(uth[:, :], utp[:B, :READ_SLOTS], AF.Tanh,
                                 scale=0.5)
            upd_sb = persist.tile([B, READ_SLOTS], F32, tag="upd")
            nc.vector.tensor_scalar(upd_sb[:, :], uth[:, :], 0.5, 0.5,
                                    op0=ALU.mult, op1=ALU.add)
            # replicate to [48, 1] (i-major) for the batched r2 update
            u48 = persist.tile([P3, 1], F32, tag="u48")
            for i in range(READ_SLOTS):
                nc.sync.dma_start(out=u48[i * B:(i + 1) * B, :],
                                  in_=upd_sb[:, i:i + 1])

            # ---- batched update branch: upd48 = relu(FA_um(cat3)@W_um + b_um)
            f_um = _emit_fa(nc, pools, cat3, P3, CU, umw1, umb1, umw2, umb2,
                            identity, ones, "um", mm_dt=BF16)
            fT = _emit_transposes(nc, tpp, persist, f_um, P3, CU, identity,
                                  "fT", out_dtype=BF16)
            upd48 = persist.tile([P3, SLOT_SIZE], F32, tag="upd48")
            tp48 = psum.tile([48, MMF], F32, tag="mm")
            _mm_bias(nc, tp48, fT, wum, P3, 0, SLOT_SIZE, ones, bumr)
            nc.scalar.activation(upd48[:, :], tp48[:P3, :SLOT_SIZE], AF.Relu)

            # r2 = r + u * (upd - r)
            d48 = persist.tile([P3, SLOT_SIZE], F32, tag="d48")
            nc.vector.tensor_sub(d48[:, :], upd48[:, :], cat3[:, :SLOT_SIZE])
            r2 = persist.tile([P3, SLOT_SIZE], F32, tag="r2")
            nc.vector.scalar_tensor_tensor(r2[:, :], d48[:, :], u48[:, :],
                                           cat3[:, :SLOT_SIZE],
                                           op0=ALU.mult, op1=ALU.add)

            # r2^T k-tiles [128, 48]: step i slices columns 16i:16(i+1).
            r2T = _emit_transposes(nc, tpp, persist, r2, P3, SLOT_SIZE,
                                   identity, "r2T", out_dtype=F32R)

            # Per-step catT [128, 4, 16] (transposed [r_i | m] k-tiles).
            catT = []
            for i in range(READ_SLOTS):
                ct = persist.tile([128, 4, B], F32R, tag=f"catT{i}")
                for k in range(2):
                    nc.vector.tensor_copy(ct[:, k, :],
                                          r2T[k][:, i * B:(i + 1) * B])
                catT.append(ct)

            # Pre-accumulate each step's y PSUM off-chain: r-part + b1 via
            # ones-row.  The m-part matmuls (start=False) join in-chain.
            y_ps = []
            for i in range(READ_SLOTS):
                ps = ypsum.tile([B, CA], F32, tag="yam")
                for k in range(2):
                    nc.tensor.matmul(ps[:, :], r2T[k][:, i * B:(i + 1) * B],
                                     amw1[:, k, :], start=(k == 0), stop=False)
                nc.tensor.matmul(ps[:, :], ones[:, :B], amb1[:, :],
                                 start=False, stop=(i == 0))
                y_ps.append(ps)

            # ---- sequential apply chain over the 3 read slots ----
            m = None                         # step 0: m == 0
            for i in range(READ_SLOTS):
                ct = catT[i]
                ps_y = y_ps[i]
                if m is None:
                    nc.vector.memset(ct[:, 2:4, :].bitcast(F32), 0.0)
                else:
                    # m^T into catT k=2,3 (one psum group, one copy)
                    pst = tpp.tile([128, 6, 48], F32, tag="tps")
                    for k in range(2):
                        nc.tensor.transpose(
                            pst[:, k, :B], m[:, k * 128:(k + 1) * 128],
                            identity[:B, :B])
                    nc.vector.tensor_copy(ct[:, 2:4, :], pst[:, 0:2, :B])
                    for k in range(2):
                        nc.tensor.matmul(ps_y[:, :], ct[:, k + 2, :],
                                         amw1[:, k + 2, :],
                                         start=False, stop=(k == 1))
                e = persist.tile([B, CA], F32, tag=f"e{i}")
                nc.scalar.activation(e[:, :], ps_y[:, :], AF.Exp)
                ssum = persist.tile([B, 1], F32, tag=f"es{i}")
                nc.vector.reduce_sum(out=ssum[:, :], in_=e[:, :], axis=AX.X)
                rs = persist.tile([B, 1], F32, tag=f"ers{i}")
                nc.vector.reciprocal(rs[:, :], ssum[:, :])
                eT = _emit_transposes(nc, tpp, persist, e, B, CA, identity,
                                      f"eT{i}", out_dtype=F32R, alt=1)
                # g = rs*(e@amw2) + amb2   [16, 512]
                g = persist.tile([B, CA], F32, tag=f"g{i}")
                ps = psum.tile([48, MMF], F32, tag="mm")
                for k in range(4):
                    nc.tensor.matmul(ps[:B, :], eT[k], amw2[:, k, :],
                                     start=(k == 0), stop=(k == 3))
                nc.vector.scalar_tensor_tensor(
                    g[:, :], ps[:B, :], rs[:, :], amb2[:, :],
                    op0=ALU.mult, op1=ALU.add)
                # fT = catT * gT  (transposed layout; no row-layout f needed)
                gT_ps = tpp.tile([128, 6, 48], F32, tag="tps")
                for k in range(4):
                    nc.tensor.transpose(gT_ps[:, k, :B],
                                        g[:, k * 128:(k + 1) * 128],
                                        identity[:B, :B])
                # multiply straight from PSUM (skip the gT SBUF landing)
                fTt = persist.tile([128, 4, B], F32R, tag=f"fTt{i}")
                nc.vector.tensor_mul(fTt[:, :, :], gT_ps[:, 0:4, :B],
                                     ct[:, :, :].bitcast(F32))
                # m = read_w_i * relu(f@W_am + b_am) == relu(read_w_i*(..))
                ps_t = psum.tile([48, MMF], F32, tag="mm")
                _mm_bias(nc, ps_t, [fTt[:, k, :] for k in range(4)], wam,
                         B, 0, SLOT_SIZE, ones, bamr)
                m = persist.tile([B, SLOT_SIZE], F32, tag=f"m{i}")
                nc.scalar.activation(m[:, :], ps_t[:B, :SLOT_SIZE], AF.Relu,
                                     scale=readw_sb[:, i:i + 1])

            out_sb = persist.tile([B, SLOT_SIZE], F32, tag="out")
            nc.scalar.activation(out_sb[:, :], m[:, :], AF.Tanh)
            nc.sync.dma_start(out=out_dram[:, :], in_=out_sb[:, :])

    nc.compile()
    return nc


_CACHE = {}


def _get_kernels():
    if "l1" not in _CACHE:
        _CACHE["l1"] = _build_launch1()
        _CACHE["l2"] = _build_launch2()
    return _CACHE["l1"], _CACHE["l2"]


def kernel(**inputs):
    inp = {k: np.ascontiguousarray(np.asarray(v, dtype=np.float32))
           for k, v in inputs.items()}
    nc1, nc2 = _get_kernels()

    bf = ml_dtypes.bfloat16
    common = {
        "x": inp["inputs"],
        "rw1": inp["fa_r_w1"], "rw2": inp["fa_r_w2"],
        "bpk": np.concatenate([inp["fa_r_b1"], inp["fa_r_b2"]]).reshape(1, -1),
    }
    wread_bf = inp["W_read"].astype(bf)
    in_maps = []
    for c in range(N_CORES):
        m = dict(common)
        m["wsh"] = np.ascontiguousarray(wread_bf[:, c * SHARD:(c + 1) * SHARD])
        m["bsh"] = np.ascontiguousarray(inp["b_read"][c * SHARD:(c + 1) * SHARD])
        in_maps.append(m)
    res1 = bass_utils.run_bass_kernel_spmd(nc1, in_maps,
                                           core_ids=list(range(N_CORES)))

    # ---- host glue: argmax combine (+ exact rescore), gather ----
    z = np.concatenate([res1.results[c]["z"] for c in range(N_CORES)], axis=1)
    a = res1.results[0]["a"].astype(np.float64)
    read = z.reshape(B, READ_SLOTS, SLOTS)
    k = 8
    cand = np.argpartition(-read, k, axis=2)[:, :, :k]          # [B, 3, k]
    wr = inp["W_read"].reshape(IN_CH, READ_SLOTS, SLOTS)
    br = inp["b_read"].reshape(READ_SLOTS, SLOTS)
    exact = np.empty((B, READ_SLOTS, k), dtype=np.float64)
    for b in range(B):
        for i in range(READ_SLOTS):
            c = cand[b, i]
            exact[b, i] = a[b] @ wr[:, i, c].astype(np.float64) + br[i, c]
    best = np.argmax(exact, axis=2)
    read_idx = np.take_along_axis(cand, best[..., None], axis=2)[..., 0]
    read_val = np.take_along_axis(exact, best[..., None], axis=2)[..., 0]
    read_w = np.tanh(read_val).astype(np.float32)
    r_all = inp["memory"][np.arange(B)[:, None], read_idx]       # [B, 3, 256]

    bpk2 = np.concatenate([
        inp["fa_u_b1"], inp["fa_u_b2"], inp["fa_um_b1"], inp["fa_um_b2"],
        inp["b_um"], inp["fa_am_b1"], inp["fa_am_b2"], inp["b_am"],
        inp["b_uw"], np.zeros(1, np.float32)]).reshape(1, -1)
    in_map2 = {
        "x": inp["inputs"], "rall": np.ascontiguousarray(r_all),
        "readw": read_w, "bpk": bpk2,
        "uw1": inp["fa_u_w1"].astype(bf),
        "uw2": inp["fa_u_w2"].astype(bf),
        "wuw": inp["W_uw"],
        "umw1": inp["fa_um_w1"].astype(bf),
        "umw2": inp["fa_um_w2"].astype(bf),
        "wum": inp["W_um"].astype(bf),
        "amw1": inp["fa_am_w1"],
        "amw2": inp["fa_am_w2"],
        "wam": inp["W_am"],
    }
    res2 = bass_utils.run_bass_kernel_spmd(nc2, [in_map2], core_ids=[0])
    return res2.results[0]["out"]

